# revision 32
# baseline (speedup 1.0000x reference)
"""Trainium2 Bass kernel for the 3-layer LIF spiking network (nn_LIFSNN).

Contract: kernel(**inputs) takes FULL unsharded inputs (as in
reference.setup_inputs()) and returns the FULL [128, 20] output.

Strategy (8 NeuronCores, data-parallel over batch B=128 -> 16/core):
  host:   x feature-major fp32 [J, col], col = t*16+b; W1 fp32r;
          W2.T/2 and Wr.T/2 in bf16 (spikes ship to the next layer as
          S' = sign(U-1) in {-1,0,1}; (W/2)@S' = W@S - const and the
          const cancels in the training-mode BatchNorm);
          sigmoid(beta); pack small params.
  device: L1: fp32r matmul -> y1 fp32 (DRAM spill) + BN partial sums
              -> split AllReduce(8) (blocks 0-14 early, block 15 late)
              -> folded BN scale/bias
              -> explicit 512-step LIF scan, 3 DVE ops/step:
                   U[t%4] = W * (-beta); U += yhat;
                   W = (U > 1) - U   (fused scalar_tensor_tensor)
                 ACT materializes spikes once per 4 steps:
                   S'[4 cols] = Sign(U[0..3] - 1) -> bf16 buffer
              L2 matmul blocks (bf16) interleave with the scan on PE;
              psum drains (ACT) deferred one block to never stall Sign.
          L2: same -> S2'.
          readout: bf16 matmul -> BN -> 16 native tensor_tensor_scans
              -> softmax over the 20 channels: exp (fp16); PE
              column-sum; Z rows packed into one [16,512] psum via PE
              accumulation -> single reciprocal; PE broadcast -> T-sum
              -> [20, 16] per core.
  host:   gather per-core outputs -> [128, 20].
"""
import sys, os, time

sys.path.insert(0, "/opt/trn_rl_repo")

import numpy as np
import ml_dtypes

import concourse.bass as bass
import concourse.mybir as mybir
import concourse.tile as tile
import concourse.bacc as bacc
from concourse.bass_utils import run_bass_kernel_spmd

AOT = mybir.AluOpType
AFT = mybir.ActivationFunctionType
F32 = mybir.dt.float32
F32R = mybir.dt.float32r
F16 = mybir.dt.float16
BF16 = mybir.dt.bfloat16

NC = 8          # cores
B, T, J, H, O = 128, 512, 700, 512, 20
BL = B // NC    # 16 batch per core
NCOL = T * BL   # 8192 columns, col = t*BL + b
NCHUNK = H // 128   # 4 channel chunks
NBLK = NCOL // 512  # 16 column blocks of 512 (32 timesteps each)
TBLK = T // NBLK    # 32 timesteps per block
KJ = [(k * 128, min(128, J - k * 128)) for k in range((J + 127) // 128)]  # 6
NTOT = float(B * T)  # BN sample count (global)
EPS = 1e-5
NPAR = 8        # U ping-pong depth == Sign batching factor

_CACHE = {}


def _build_program():
    nc = bacc.Bacc("TRN2", target_bir_lowering=False, debug=False,
                   num_devices=NC)
    ap = lambda name, shape, dt, kind: nc.dram_tensor(name, shape, dt, kind=kind).ap()

    xT = ap("xT", [J, NCOL], F32R, "ExternalInput")
    w1t = ap("w1t", [J, H], F32R, "ExternalInput")         # W1.T
    w2t = ap("w2t", [H, H], BF16, "ExternalInput")         # (W2/2).T bf16
    wrt = ap("wrt", [H, O], BF16, "ExternalInput")         # (Wr/2).T bf16
    nb64_1 = ap("nb64_1", [128, 64], F32, "ExternalInput")  # -beta1 bcast
    nb64_2 = ap("nb64_2", [128, 64], F32, "ExternalInput")
    G1 = ap("G1", [128, NCHUNK], F32, "ExternalInput")     # g1*(1-beta1)
    Bb1 = ap("Bb1", [128, NCHUNK], F32, "ExternalInput")   # b1*(1-beta1)
    G2 = ap("G2", [128, NCHUNK], F32, "ExternalInput")
    Bb2 = ap("Bb2", [128, NCHUNK], F32, "ExternalInput")
    brb = ap("brb", [O, 512], F32, "ExternalInput")        # beta_r bcast
    Gr = ap("Gr", [O, 1], F32, "ExternalInput")
    Bbr = ap("Bbr", [O, 1], F32, "ExternalInput")
    epack_d = ap("epack", [1, NBLK * NBLK], F16, "ExternalInput")
    sel_d = ap("sel", [NBLK, NBLK * O], F16, "ExternalInput")
    out_d = ap("out", [O, BL], F32, "ExternalOutput")

    with tile.TileContext(nc) as tc:
        import contextlib
        with contextlib.ExitStack() as ctx:
            pool = ctx.enter_context(tc.tile_pool(name="main", bufs=1))
            xpool = ctx.enter_context(tc.tile_pool(name="xs", bufs=6))
            stage = ctx.enter_context(tc.tile_pool(name="stage", bufs=2))
            ybufp = ctx.enter_context(tc.tile_pool(name="ybufp", bufs=4))
            ps1 = ctx.enter_context(tc.tile_pool(name="ps1", bufs=2, space="PSUM"))
            dram = ctx.enter_context(tc.tile_pool(name="dram", bufs=1, space="DRAM"))
            scr = ctx.enter_context(tc.tile_pool(name="scr", bufs=2))

            # ---- DRAM scratch for y (fp32)
            y_dram = [dram.tile([128, NCHUNK, NCOL], F32, tag=f"ydram{i}", name=f"ydram{i}")
                      for i in range(2)]

            # ---- persistent SBUF
            S = pool.tile([128, NCHUNK, NCOL], BF16, tag="S", name="S")  # spikes
            y3 = pool.tile([O, NCOL], F32, tag="y3", name="y3")
            w1_sb = [[pool.tile([kk, 128], F32R, tag=f"w1_{k}_{m}", name=f"w1_{k}_{m}")
                      for m in range(NCHUNK)] for k, (k0, kk) in enumerate(KJ)]
            for k, (k0, kk) in enumerate(KJ):
                for m in range(NCHUNK):
                    nc.sync.dma_start(w1_sb[k][m][:], w1t[k0:k0 + kk, m * 128:(m + 1) * 128])
            # (w2/wr/param DMAs are issued after the L1 loop emission so the
            # sync queue serves the first x blocks immediately)
            w2_sb = [[pool.tile([128, 128], BF16, tag=f"w2_{k}_{m}", name=f"w2_{k}_{m}")
                      for m in range(NCHUNK)] for k in range(NCHUNK)]
            wr_sb = [pool.tile([128, O], BF16, tag=f"wr_{k}", name=f"wr_{k}") for k in range(NCHUNK)]
            nbl = [pool.tile([128, 64], F32, tag=f"nb64_{i}", name=f"nb64_{i}") for i in range(2)]
            Gl = [pool.tile([128, NCHUNK], F32, tag=f"G_{i}", name=f"G_{i}") for i in range(2)]
            Bbl = [pool.tile([128, NCHUNK], F32, tag=f"Bb_{i}", name=f"Bb_{i}") for i in range(2)]
            brb_sb = pool.tile([O, 512], F32, tag="brb", name="brb")
            Gr_sb = pool.tile([O, 1], F32, tag="Gr", name="Gr")
            Bbr_sb = pool.tile([O, 1], F32, tag="Bbr", name="Bbr")

            def load_params():
                for k in range(NCHUNK):
                    for m in range(NCHUNK):
                        nc.sync.dma_start(w2_sb[k][m][:], w2t[k * 128:(k + 1) * 128, m * 128:(m + 1) * 128])
                for k in range(NCHUNK):
                    nc.sync.dma_start(wr_sb[k][:], wrt[k * 128:(k + 1) * 128, :])
                nc.sync.dma_start(nbl[0][:], nb64_1)
                nc.sync.dma_start(nbl[1][:], nb64_2)
                nc.sync.dma_start(Gl[0][:], G1); nc.sync.dma_start(Bbl[0][:], Bb1)
                nc.sync.dma_start(Gl[1][:], G2); nc.sync.dma_start(Bbl[1][:], Bb2)
                nc.sync.dma_start(brb_sb[:], brb)
                nc.sync.dma_start(Gr_sb[:], Gr); nc.sync.dma_start(Bbr_sb[:], Bbr)

            sums = [pool.tile([128, NCHUNK, NBLK], F32, tag=f"sums{i}", name=f"sums{i}") for i in range(2)]
            sqs = [pool.tile([128, NCHUNK, NBLK], F32, tag=f"sqs{i}", name=f"sqs{i}") for i in range(2)]
            sumr = pool.tile([O, NBLK], F32, tag="sumr", name="sumr")
            sqr = pool.tile([O, NBLK], F32, tag="sqr", name="sqr")

            # scan state: U has NPAR ping-pong slots (chunk-major so the
            # batched ACT Sign reads one contiguous run) so the Sign instr
            # (one per NPAR steps) never blocks the DVE recurrence.
            Uq = pool.tile([128, NCHUNK, NPAR, BL], F32, tag="Uq", name="Uq")
            Wt = pool.tile([128, NCHUNK, BL], F32, tag="Wst", name="Wst")
            negone = pool.tile([128, 1], F32, tag="negone", name="negone")
            nc.gpsimd.memset(negone[:], -1.0)

            # ---------------- helpers ----------------
            def bn_coeffs(li, stats_sb, nch, parts, Gt, Bbt):
                m_ = pool.tile([parts, nch], F32, tag=f"mean{li}", name=f"mean{li}")
                v_ = pool.tile([parts, nch], F32, tag=f"var{li}", name=f"var{li}")
                t_ = pool.tile([parts, nch], F32, tag=f"tmp{li}", name=f"tmp{li}")
                c0 = pool.tile([parts, nch], F32, tag=f"c0_{li}", name=f"c0_{li}")
                c1 = pool.tile([parts, nch], F32, tag=f"c1_{li}", name=f"c1_{li}")
                inv_n = 1.0 / NTOT
                nc.vector.tensor_scalar(m_[:], stats_sb[:, 0:nch], inv_n, None, AOT.mult)
                nc.vector.tensor_scalar(v_[:], stats_sb[:, nch:2 * nch], inv_n, None, AOT.mult)
                nc.vector.tensor_tensor(t_[:], m_[:], m_[:], AOT.mult)
                nc.vector.tensor_tensor(v_[:], v_[:], t_[:], AOT.subtract)
                nc.vector.tensor_scalar(v_[:], v_[:], EPS, None, AOT.add)
                nc.scalar.sqrt(v_[:], v_[:])
                nc.vector.reciprocal(v_[:], v_[:])      # 1/sqrt(var+eps)
                nc.vector.tensor_tensor(c0[:], v_[:], Gt[:], AOT.mult)
                nc.vector.tensor_tensor(t_[:], m_[:], c0[:], AOT.mult)
                nc.vector.tensor_tensor(c1[:], Bbt[:], t_[:], AOT.subtract)
                return c0, c1

            def allreduce(sb_tile, parts, width, tag):
                din = dram.tile([parts, width], F32, tag=f"cin{tag}", name=f"cin{tag}")
                dout = dram.tile([parts, width], F32, tag=f"cout{tag}", name=f"cout{tag}")
                g = pool.tile([parts, width], F32, tag=f"gst{tag}", name=f"gst{tag}")
                nc.sync.dma_start(din[:], sb_tile[:])
                nc.gpsimd.collective_compute(
                    "AllReduce", AOT.add,
                    replica_groups=[list(range(NC))],
                    ins=[din.opt()], outs=[dout.opt()],
                )
                nc.sync.dma_start(g[:], dout[:])
                return g

            def stats_ar(li, sub, parts, nch, sums_t, sqs_t, lo, hi):
                st = pool.tile([parts, 2 * nch], F32, tag=f"st{sub}{li}", name=f"st{sub}{li}")
                nc.vector.tensor_reduce(st[:, 0:nch],
                                        sums_t[:, :, lo:hi] if nch > 1 else sums_t[:, lo:hi],
                                        mybir.AxisListType.X, AOT.add)
                nc.vector.tensor_reduce(st[:, nch:],
                                        sqs_t[:, :, lo:hi] if nch > 1 else sqs_t[:, lo:hi],
                                        mybir.AxisListType.X, AOT.add)
                return allreduce(st, parts, 2 * nch, f"{sub}{li}")

            def finish_stats(li, parts, nch, ga, gb, Gt, Bbt):
                g = pool.tile([parts, 2 * nch], F32, tag=f"g{li}", name=f"g{li}")
                nc.vector.tensor_tensor(g[:], ga[:], gb[:], AOT.add)
                return bn_coeffs(li, g, nch, parts, Gt, Bbt)

            NSPLIT = 15  # AR_a covers stat blocks [0,15), AR_b block 15

            _yb = {}

            def prefetch_y(li, nn):
                t_ = ybufp.tile([128, NCHUNK, 512], F32, tag="ybuf", name="ybuf")
                nc.sync.dma_start(t_[:], y_dram[li][:, :, nn * 512:(nn + 1) * 512])
                _yb[nn] = t_

            # ================ LAYER 1 matmul ================
            ar1a = None
            for n in range(NBLK):
                cols = slice(n * 512, (n + 1) * 512)
                psl = [ps1.tile([128, 512], F32, tag=f"ps_m{m}", name=f"ps_m{m}")
                       for m in range(NCHUNK)]
                rhs_tiles = []
                for k, (k0, kk) in enumerate(KJ):
                    xt_t = xpool.tile([128, 512], F32R, tag="xstream", name="xstream")
                    nc.sync.dma_start(xt_t[:kk, :], xT[k0:k0 + kk, cols])
                    rhs_tiles.append(xt_t[:kk, :])
                for k in range(len(KJ)):
                    for m in range(NCHUNK):
                        nc.tensor.matmul(psl[m][:], w1_sb[k][m][:], rhs_tiles[k],
                                         start=(k == 0), stop=(k == len(KJ) - 1))
                st_t = stage.tile([128, NCHUNK, 512], F32, tag="ystage", name="ystage")
                for m in range(NCHUNK):
                    nc.scalar.activation(st_t[:, m, :], psl[m][:], AFT.Copy,
                                         accum_out=sums[0][:, m, n:n + 1])
                    sc = scr.tile([128, 512], BF16, tag="sq_scratch", name="sq_scratch")
                    nc.scalar.activation(sc[:], psl[m][:], AFT.Square,
                                         accum_out=sqs[0][:, m, n:n + 1])
                nc.sync.dma_start(y_dram[0][:, :, cols], st_t[:])
                if n == 0:
                    load_params()
                if n < 3:
                    prefetch_y(0, n)
                if n == NSPLIT - 1:
                    ar1a = stats_ar(0, "a", 128, NCHUNK, sums[0], sqs[0], 0, NSPLIT)
            ar1b = stats_ar(0, "b", 128, NCHUNK, sums[0], sqs[0], NSPLIT, NBLK)
            c0_1, c1_1 = finish_stats(0, 128, NCHUNK, ar1a, ar1b, Gl[0], Bbl[0])

            # ---------------- LIF scan ----------------
            def fold_chunk(nn, m, c0, c1):
                """In place: yb[m] = c0 * y[m] + c1 (one ACT instr)."""
                yb = _yb[nn]
                nc.scalar.activation(yb[:, m, :], yb[:, m, :], AFT.Identity,
                                     bias=c1[:, m:m + 1], scale=c0[:, m:m + 1])

            # every ACT insert between Signs is a single instruction, so the
            # NPAR-step WAR window always absorbs it; drain j (chunk j//2 of
            # the m-major interleaved matmul) lands after its psum completes
            DRAIN_TT = {6: 0, 8: 1, 12: 2, 14: 3, 18: 4, 20: 5, 24: 6, 26: 7}
            FOLD_TT = {3: 0, 10: 1, 16: 2, 22: 3}

            def lif_scan(li, c0, c1, mm_cb, drain_cb):
                """512-step LIF scan; 3 DVE ops/step x 2 interleaved chains,
                Sign batched per NPAR steps on ACT.

                mm_cb(n): next-layer matmuls for block n (PE queue).
                drain_cb(n, j): j-th psum drain sub-step for block n (ACT),
                spread one instruction at a time through block n+1.
                """
                nbv = nbl[li][:].rearrange("p (c b) -> p c b", c=NCHUNK)
                nc.gpsimd.memset(Wt[:], 0.0)
                for m in range(NCHUNK):
                    fold_chunk(0, m, c0, c1)
                for m in range(NCHUNK):
                    fold_chunk(1, m, c0, c1)
                for n in range(NBLK):
                    if n + 3 < NBLK:
                        prefetch_y(li, n + 3)
                    yb = _yb.pop(n)
                    for tt in range(TBLK):
                        t = n * TBLK + tt
                        p = t % NPAR
                        bs = slice(tt * BL, (tt + 1) * BL)
                        # two independent chunk-chains interleaved so the
                        # DVE pipelines (no back-to-back RAW drain stalls)
                        Ua = Uq[:, 0:2, p, :]
                        Ub = Uq[:, 2:4, p, :]
                        nc.vector.tensor_tensor(Ua, Wt[:, 0:2, :], nbv[:, 0:2, :], AOT.mult)
                        nc.vector.tensor_tensor(Ub, Wt[:, 2:4, :], nbv[:, 2:4, :], AOT.mult)
                        nc.vector.tensor_tensor(Ua, Ua, yb[:, 0:2, bs], AOT.add)
                        nc.vector.tensor_tensor(Ub, Ub, yb[:, 2:4, bs], AOT.add)
                        nc.vector.scalar_tensor_tensor(Wt[:, 0:2, :], Ua, 1.0, Ua,
                                                       AOT.is_gt, AOT.subtract)
                        nc.vector.scalar_tensor_tensor(Wt[:, 2:4, :], Ub, 1.0, Ub,
                                                       AOT.is_gt, AOT.subtract)
                        if p == NPAR - 1:
                            # S'[cols t-7..t] = Sign(U[0..7] - 1), one ACT op
                            # (contiguous 128-col run per chunk on both sides)
                            t0 = t - (NPAR - 1)
                            sdst = S[:, :, t0 * BL:(t0 + NPAR) * BL]
                            usrc = Uq[:].rearrange("p c q b -> p c (q b)")
                            nc.scalar.activation(sdst, usrc, AFT.Sign,
                                                 bias=negone[:])
                        if drain_cb is not None and n > 0 and tt in DRAIN_TT:
                            drain_cb(n - 1, DRAIN_TT[tt])
                        if n + 2 < NBLK and tt in FOLD_TT:
                            fold_chunk(n + 2, FOLD_TT[tt], c0, c1)
                    if mm_cb is not None:
                        mm_cb(n)
                if drain_cb is not None:
                    for j in range(8):
                        drain_cb(NBLK - 1, j)

            # scan1 + interleaved L2 matmul
            _l2ps = {}

            def l2_mm_cb(n):
                cols = slice(n * 512, (n + 1) * 512)
                psl = [ps1.tile([128, 512], F32, tag=f"ps_m{m}", name=f"ps_m{m}")
                       for m in range(NCHUNK)]
                for m in range(NCHUNK):  # m-major: chunk psums finish early
                    for k in range(NCHUNK):
                        nc.tensor.matmul(psl[m][:], w2_sb[k][m][:], S[:, k, cols],
                                         start=(k == 0), stop=(k == NCHUNK - 1))
                _l2ps[n] = psl

            ar2a = None
            _l2st = {}

            def l2_drain_cb(n, j):
                nonlocal ar2a
                m = j // 2
                if j == 0:
                    _l2st[n] = stage.tile([128, NCHUNK, 512], F32, tag="ystage", name="ystage")
                st_t = _l2st[n]
                if j % 2 == 0:
                    nc.scalar.activation(st_t[:, m, :], _l2ps[n][m][:], AFT.Copy,
                                         accum_out=sums[1][:, m, n:n + 1])
                else:
                    sc = scr.tile([128, 512], BF16, tag="sq_scratch", name="sq_scratch")
                    nc.scalar.activation(sc[:], _l2ps[n][m][:], AFT.Square,
                                         accum_out=sqs[1][:, m, n:n + 1])
                if j == 7:
                    _l2ps.pop(n)
                    _l2st.pop(n)
                    nc.sync.dma_start(y_dram[1][:, :, n * 512:(n + 1) * 512], st_t[:])
                    if n < 3:
                        prefetch_y(1, n)
                    if n == NSPLIT - 1:
                        ar2a = stats_ar(1, "a", 128, NCHUNK, sums[1], sqs[1], 0, NSPLIT)

            lif_scan(0, c0_1, c1_1, l2_mm_cb, l2_drain_cb)
            ar2b = stats_ar(1, "b", 128, NCHUNK, sums[1], sqs[1], NSPLIT, NBLK)
            c0_2, c1_2 = finish_stats(1, 128, NCHUNK, ar2a, ar2b, Gl[1], Bbl[1])

            # scan2 + interleaved readout matmul
            _l3ps = {}

            def l3_mm_cb(n):
                cols = slice(n * 512, (n + 1) * 512)
                ps = ps1.tile([O, 512], F32, tag="ps_m0", name="ps_r")
                for k in range(NCHUNK):
                    nc.tensor.matmul(ps[:], wr_sb[k][:], S[:, k, cols],
                                     start=(k == 0), stop=(k == NCHUNK - 1))
                _l3ps[n] = ps

            ar3a = None

            def l3_drain_cb(n, j):
                nonlocal ar3a
                cols = slice(n * 512, (n + 1) * 512)
                if j == 0:
                    nc.scalar.activation(y3[:, cols], _l3ps[n][:], AFT.Copy,
                                         accum_out=sumr[:, n:n + 1])
                elif j == 1:
                    sc = scr.tile([O, 512], BF16, tag="sq3_scratch", name="sq3_scratch")
                    nc.scalar.activation(sc[:], _l3ps[n][:], AFT.Square,
                                         accum_out=sqr[:, n:n + 1])
                    _l3ps.pop(n)
                    if n == NSPLIT - 1:
                        ar3a = stats_ar(2, "a", O, 1, sumr, sqr, 0, NSPLIT)

            lif_scan(1, c0_2, c1_2, l3_mm_cb, l3_drain_cb)
            ar3b = stats_ar(2, "b", O, 1, sumr, sqr, NSPLIT, NBLK)
            c0_r, c1_r = finish_stats(2, O, 1, ar3a, ar3b, Gr_sb, Bbr_sb)

            # ================ READOUT ================
            # BN-fold y3 in place (4 slabs for pipelining)
            for q in range(4):
                sl = slice(q * 2048, (q + 1) * 2048)
                nc.scalar.activation(y3[:, sl], y3[:, sl], AFT.Identity,
                                     bias=c1_r[:, 0:1], scale=c0_r[:, 0:1])
            # leaky-integrator scans, in place (fp32)
            y3v = y3[:].rearrange("p (t b) -> p t b", b=BL)
            for b in range(BL):
                sl = y3v[:, :, b]
                nc.vector.tensor_tensor_scan(sl, brb_sb[:], sl, 0.0, AOT.mult, AOT.add)

            # softmax over channels, in place; then T-sum.
            # Phase 1: Z rows for all 16 blocks packed into one [16,512]
            # psum via PE accumulation; single exact reciprocal.
            ones_k20 = pool.tile([O, 1], F16, tag="ones_k20", name="ones_k20")
            nc.gpsimd.memset(ones_k20[:], 1.0)
            # E[0, n*16+m] = (n == m): row selectors for the Z-pack matmuls
            Epack = pool.tile([1, NBLK * NBLK], F16, tag="Epack", name="Epack")
            nc.sync.dma_start(Epack[:], epack_d)
            # Sel[k, n*20:(n+1)*20] = (k == n): selects Z row n, bcast to 20
            Sel = pool.tile([NBLK, NBLK * O], F16, tag="Sel", name="Sel")
            nc.sync.dma_start(Sel[:], sel_d)
            zall_ps = ps1.tile([NBLK, 512], F32, tag="ps_m1", name="ps_zall")
            for n in range(NBLK):
                cols = slice(n * 512, (n + 1) * 512)
                En = scr.tile([O, 512], F16, tag="En", name="En")
                nc.scalar.activation(En[:], y3[:, cols], AFT.Exp)
                psz = ps1.tile([1, 512], F32, tag="ps_m2", name="ps_z")
                nc.tensor.matmul(psz[:], ones_k20[:], En[:], start=True, stop=True)
                zsb = scr.tile([1, 512], F16, tag="zsb", name="zsb")
                nc.scalar.copy(zsb[:], psz[:])
                nc.tensor.matmul(zall_ps[:], Epack[0:1, n * NBLK:(n + 1) * NBLK],
                                 zsb[:], start=(n == 0), stop=(n == NBLK - 1))
            Rall = pool.tile([NBLK, 512], F16, tag="Rall", name="Rall")
            with nc.allow_low_precision(reason="softmax denominator, fp16 ok"):
                nc.vector.reciprocal(Rall[:], zall_ps[:])
            # Phase 2: broadcast 1/Z to 20 partitions per block, multiply.
            for n in range(NBLK):
                cols = slice(n * 512, (n + 1) * 512)
                En = scr.tile([O, 512], F16, tag="En", name="En")
                nc.scalar.activation(En[:], y3[:, cols], AFT.Exp)
                psb = ps1.tile([O, 512], F32, tag="ps_m3", name="ps_b")
                nc.tensor.matmul(psb[:], Sel[:, n * O:(n + 1) * O], Rall[:],
                                 start=True, stop=True)
                nc.vector.tensor_tensor(y3[:, cols], En[:], psb[:], AOT.mult)
            # T-sum: view [O, b, t] -> reduce over t
            res = pool.tile([O, BL], F32, tag="res", name="res")
            accv = y3[:].rearrange("p (t b) -> p b t", b=BL)
            nc.vector.tensor_reduce(res[:, 0:BL // 2], accv[:, 0:BL // 2, :],
                                    mybir.AxisListType.X, AOT.add)
            nc.vector.tensor_reduce(res[:, BL // 2:], accv[:, BL // 2:, :],
                                    mybir.AxisListType.X, AOT.add)
            nc.sync.dma_start(out_d, res[:])

    nc.compile()
    return nc


def _host_prep(inputs):
    f32 = np.float32
    x = np.asarray(inputs["x"], f32)
    sig = lambda v: (1.0 / (1.0 + np.exp(-np.asarray(v, np.float64)))).astype(f32)

    def packed(vec):  # [H] -> [128, NCHUNK]
        return np.ascontiguousarray(np.asarray(vec, f32).reshape(NCHUNK, 128).T)

    beta1, beta2, betar = sig(inputs["beta1"]), sig(inputs["beta2"]), sig(inputs["betar"])

    def nbcast(beta):  # [H] -> [128, 64] = -beta, chunk-major, bcast over b
        p = packed(-beta)  # [128, 4]
        return np.ascontiguousarray(np.repeat(p[:, :, None], BL, axis=2).reshape(128, NCHUNK * BL))

    com = {
        "w1t": np.ascontiguousarray(np.asarray(inputs["W1"], f32).T),
        "w2t": np.ascontiguousarray(np.asarray(inputs["W2"], f32).T * 0.5).astype(ml_dtypes.bfloat16),
        "wrt": np.ascontiguousarray(np.asarray(inputs["Wr"], f32).T * 0.5).astype(ml_dtypes.bfloat16),
        "nb64_1": nbcast(beta1),
        "nb64_2": nbcast(beta2),
        "G1": packed(np.asarray(inputs["g1"], f32) * (1 - beta1)),
        "Bb1": packed(np.asarray(inputs["b1"], f32) * (1 - beta1)),
        "G2": packed(np.asarray(inputs["g2"], f32) * (1 - beta2)),
        "Bb2": packed(np.asarray(inputs["b2"], f32) * (1 - beta2)),
        "brb": np.ascontiguousarray(np.repeat(betar[:, None], 512, axis=1)),
        "Gr": np.ascontiguousarray((np.asarray(inputs["gr"], f32) * (1 - betar))[:, None]),
        "Bbr": np.ascontiguousarray((np.asarray(inputs["br"], f32) * (1 - betar))[:, None]),
        "epack": np.eye(NBLK, dtype=np.float16).reshape(1, NBLK * NBLK),
        "sel": np.ascontiguousarray(
            np.repeat(np.eye(NBLK, dtype=np.float16)[:, :, None], O, axis=2).reshape(NBLK, NBLK * O)),
    }
    in_maps = []
    for c in range(NC):
        xc = x[c * BL:(c + 1) * BL]              # [BL, T, J]
        xTc = np.ascontiguousarray(xc.transpose(2, 1, 0).reshape(J, NCOL))
        m = dict(com)
        m["xT"] = xTc
        in_maps.append(m)
    return in_maps


def kernel(**inputs):
    if "nc" not in _CACHE:
        _CACHE["nc"] = _build_program()
    nc = _CACHE["nc"]
    in_maps = _host_prep(inputs)
    res = run_bass_kernel_spmd(nc, in_maps, core_ids=list(range(NC)),
                               trace=bool(os.environ.get("BASS_TRACE_KERNEL")))
    _CACHE["last_result"] = res
    out = np.empty((B, O), np.float32)
    for c in range(NC):
        out[c * BL:(c + 1) * BL, :] = res.results[c]["out"].T
    return out


if __name__ == "__main__":
    t0 = time.time()
    nc = _build_program()
    print(f"build+compile ok in {time.time()-t0:.1f}s")


# revision 41
# speedup vs baseline: 1.0649x; 1.0649x over previous
"""Trainium2 Bass kernel for the 3-layer LIF spiking network (nn_LIFSNN).

Contract: kernel(**inputs) takes FULL unsharded inputs (as in
reference.setup_inputs()) and returns the FULL [128, 20] output.

Strategy (8 NeuronCores, data-parallel over batch B=128 -> 16/core):
  host:   x feature-major fp32 [J, col], col = t*16+b; W1 fp32r;
          W2.T/2 and Wr.T/2 in bf16 (spikes ship to the next layer as
          S' = sign(U-1) in {-1,0,1}; (W/2)@S' = W@S - const and the
          const cancels in the training-mode BatchNorm);
          sigmoid(beta); pack small params.
  device: L1: fp32r matmul -> y1 fp32 (DRAM spill) + BN partial sums
              -> split AllReduce(8) (blocks 0-14 early, block 15 late)
              -> folded BN scale/bias
              -> explicit 512-step LIF scan, 3 DVE ops/step:
                   U[t%4] = W * (-beta); U += yhat;
                   W = (U > 1) - U   (fused scalar_tensor_tensor)
                 ACT materializes spikes once per 4 steps:
                   S'[4 cols] = Sign(U[0..3] - 1) -> bf16 buffer
              L2 matmul blocks (bf16) interleave with the scan on PE;
              psum drains (ACT) deferred one block to never stall Sign.
          L2: same -> S2'.
          readout: bf16 matmul -> BN -> 16 native tensor_tensor_scans
              -> softmax over the 20 channels: exp (fp16); PE
              column-sum; Z rows packed into one [16,512] psum via PE
              accumulation -> single reciprocal; PE broadcast -> T-sum
              -> [20, 16] per core.
  host:   gather per-core outputs -> [128, 20].
"""
import sys, os, time

sys.path.insert(0, "/opt/trn_rl_repo")

import numpy as np
import ml_dtypes

import concourse.bass as bass
import concourse.mybir as mybir
import concourse.tile as tile
import concourse.bacc as bacc
from concourse.bass_utils import run_bass_kernel_spmd

AOT = mybir.AluOpType
AFT = mybir.ActivationFunctionType
F32 = mybir.dt.float32
F32R = mybir.dt.float32r
F16 = mybir.dt.float16
BF16 = mybir.dt.bfloat16

NC = 8          # cores
B, T, J, H, O = 128, 512, 700, 512, 20
BL = B // NC    # 16 batch per core
NCOL = T * BL   # 8192 columns, col = t*BL + b
NCHUNK = H // 128   # 4 channel chunks
NBLK = NCOL // 512  # 16 column blocks of 512 (32 timesteps each)
TBLK = T // NBLK    # 32 timesteps per block
KJ = [(k * 128, min(128, J - k * 128)) for k in range((J + 127) // 128)]  # 6
NTOT = float(B * T)  # BN sample count (global)
EPS = 1e-5
NPAR = 8        # U ping-pong depth == Sign batching factor

_CACHE = {}


def _build_program():
    nc = bacc.Bacc("TRN2", target_bir_lowering=False, debug=False,
                   num_devices=NC)
    ap = lambda name, shape, dt, kind: nc.dram_tensor(name, shape, dt, kind=kind).ap()

    xT = ap("xT", [J, NCOL], F32R, "ExternalInput")
    w1t = ap("w1t", [J, H], F32R, "ExternalInput")         # W1.T
    w2t = ap("w2t", [H, H], BF16, "ExternalInput")         # (W2/2).T bf16
    wrt = ap("wrt", [H, O], BF16, "ExternalInput")         # (Wr/2).T bf16
    nb64_1 = ap("nb64_1", [128, 64], F32, "ExternalInput")  # -beta1 bcast
    nb64_2 = ap("nb64_2", [128, 64], F32, "ExternalInput")
    G1 = ap("G1", [128, NCHUNK], F32, "ExternalInput")     # g1*(1-beta1)
    Bb1 = ap("Bb1", [128, NCHUNK], F32, "ExternalInput")   # b1*(1-beta1)
    G2 = ap("G2", [128, NCHUNK], F32, "ExternalInput")
    Bb2 = ap("Bb2", [128, NCHUNK], F32, "ExternalInput")
    brb = ap("brb", [O, 512], F32, "ExternalInput")        # beta_r bcast
    Gr = ap("Gr", [O, 1], F32, "ExternalInput")
    Bbr = ap("Bbr", [O, 1], F32, "ExternalInput")
    epack_d = ap("epack", [1, NBLK * NBLK], F16, "ExternalInput")
    sel_d = ap("sel", [NBLK, NBLK * O], F16, "ExternalInput")
    out_d = ap("out", [O, BL], F32, "ExternalOutput")

    with tile.TileContext(nc) as tc:
        import contextlib
        with contextlib.ExitStack() as ctx:
            pool = ctx.enter_context(tc.tile_pool(name="main", bufs=1))
            xpool = ctx.enter_context(tc.tile_pool(name="xs", bufs=6))
            stage = ctx.enter_context(tc.tile_pool(name="stage", bufs=2))
            ybufp = ctx.enter_context(tc.tile_pool(name="ybufp", bufs=4))
            ps1 = ctx.enter_context(tc.tile_pool(name="ps1", bufs=2, space="PSUM"))
            dram = ctx.enter_context(tc.tile_pool(name="dram", bufs=1, space="DRAM"))
            scr = ctx.enter_context(tc.tile_pool(name="scr", bufs=2))

            # ---- DRAM scratch for y (fp32)
            y_dram = [dram.tile([128, NCHUNK, NCOL], F32, tag=f"ydram{i}", name=f"ydram{i}")
                      for i in range(2)]

            # ---- persistent SBUF
            S = pool.tile([128, NCHUNK, NCOL], BF16, tag="S", name="S")  # spikes
            y3 = pool.tile([O, NCOL], F32, tag="y3", name="y3")
            w1_sb = [[pool.tile([kk, 128], F32R, tag=f"w1_{k}_{m}", name=f"w1_{k}_{m}")
                      for m in range(NCHUNK)] for k, (k0, kk) in enumerate(KJ)]
            for k, (k0, kk) in enumerate(KJ):
                for m in range(NCHUNK):
                    nc.sync.dma_start(w1_sb[k][m][:], w1t[k0:k0 + kk, m * 128:(m + 1) * 128])
            # (w2/wr/param DMAs are issued after the L1 loop emission so the
            # sync queue serves the first x blocks immediately)
            w2_sb = [[pool.tile([128, 128], BF16, tag=f"w2_{k}_{m}", name=f"w2_{k}_{m}")
                      for m in range(NCHUNK)] for k in range(NCHUNK)]
            wr_sb = [pool.tile([128, O], BF16, tag=f"wr_{k}", name=f"wr_{k}") for k in range(NCHUNK)]
            nbl = [pool.tile([128, 64], F32, tag=f"nb64_{i}", name=f"nb64_{i}") for i in range(2)]
            Gl = [pool.tile([128, NCHUNK], F32, tag=f"G_{i}", name=f"G_{i}") for i in range(2)]
            Bbl = [pool.tile([128, NCHUNK], F32, tag=f"Bb_{i}", name=f"Bb_{i}") for i in range(2)]
            brb_sb = pool.tile([O, 512], F32, tag="brb", name="brb")
            Gr_sb = pool.tile([O, 1], F32, tag="Gr", name="Gr")
            Bbr_sb = pool.tile([O, 1], F32, tag="Bbr", name="Bbr")

            def load_params():
                for k in range(NCHUNK):
                    for m in range(NCHUNK):
                        nc.sync.dma_start(w2_sb[k][m][:], w2t[k * 128:(k + 1) * 128, m * 128:(m + 1) * 128])
                for k in range(NCHUNK):
                    nc.sync.dma_start(wr_sb[k][:], wrt[k * 128:(k + 1) * 128, :])
                nc.sync.dma_start(nbl[0][:], nb64_1)
                nc.sync.dma_start(nbl[1][:], nb64_2)
                nc.sync.dma_start(Gl[0][:], G1); nc.sync.dma_start(Bbl[0][:], Bb1)
                nc.sync.dma_start(Gl[1][:], G2); nc.sync.dma_start(Bbl[1][:], Bb2)
                nc.sync.dma_start(brb_sb[:], brb)
                nc.sync.dma_start(Gr_sb[:], Gr); nc.sync.dma_start(Bbr_sb[:], Bbr)

            sums = [pool.tile([128, NCHUNK, NBLK], F32, tag=f"sums{i}", name=f"sums{i}") for i in range(2)]
            sqs = [pool.tile([128, NCHUNK, NBLK], F32, tag=f"sqs{i}", name=f"sqs{i}") for i in range(2)]
            sumr = pool.tile([O, NBLK], F32, tag="sumr", name="sumr")
            sqr = pool.tile([O, NBLK], F32, tag="sqr", name="sqr")

            # scan state: U has NPAR ping-pong slots (chunk-major so the
            # batched ACT Sign reads one contiguous run) so the Sign instr
            # (one per NPAR steps) never blocks the DVE recurrence.
            Uq = pool.tile([128, NCHUNK, NPAR, BL], F32, tag="Uq", name="Uq")
            Wt = pool.tile([128, NCHUNK, BL], F32, tag="Wst", name="Wst")
            negone = pool.tile([128, 1], F32, tag="negone", name="negone")
            nc.gpsimd.memset(negone[:], -1.0)

            # ---------------- helpers ----------------
            def bn_coeffs(li, stats_sb, nch, parts, Gt, Bbt):
                m_ = pool.tile([parts, nch], F32, tag=f"mean{li}", name=f"mean{li}")
                v_ = pool.tile([parts, nch], F32, tag=f"var{li}", name=f"var{li}")
                t_ = pool.tile([parts, nch], F32, tag=f"tmp{li}", name=f"tmp{li}")
                c0 = pool.tile([parts, nch], F32, tag=f"c0_{li}", name=f"c0_{li}")
                c1 = pool.tile([parts, nch], F32, tag=f"c1_{li}", name=f"c1_{li}")
                inv_n = 1.0 / NTOT
                nc.vector.tensor_scalar(m_[:], stats_sb[:, 0:nch], inv_n, None, AOT.mult)
                nc.vector.tensor_scalar(v_[:], stats_sb[:, nch:2 * nch], inv_n, None, AOT.mult)
                nc.vector.tensor_tensor(t_[:], m_[:], m_[:], AOT.mult)
                nc.vector.tensor_tensor(v_[:], v_[:], t_[:], AOT.subtract)
                nc.vector.tensor_scalar(v_[:], v_[:], EPS, None, AOT.add)
                nc.scalar.sqrt(v_[:], v_[:])
                nc.vector.reciprocal(v_[:], v_[:])      # 1/sqrt(var+eps)
                nc.vector.tensor_tensor(c0[:], v_[:], Gt[:], AOT.mult)
                nc.vector.tensor_tensor(t_[:], m_[:], c0[:], AOT.mult)
                nc.vector.tensor_tensor(c1[:], Bbt[:], t_[:], AOT.subtract)
                return c0, c1

            def allreduce(sb_tile, parts, width, tag):
                # din rides the ACT queue: the sync DMA queue must never
                # head-block on the collective (y prefetches flow through
                # it). The result read is deferred to finish_stats so the
                # ACT queue doesn't head-block on the collective either.
                din = dram.tile([parts, width], F32, tag=f"cin{tag}", name=f"cin{tag}")
                dout = dram.tile([parts, width], F32, tag=f"cout{tag}", name=f"cout{tag}")
                g = pool.tile([parts, width], F32, tag=f"gst{tag}", name=f"gst{tag}")
                nc.scalar.dma_start(din[:], sb_tile[:])
                nc.gpsimd.collective_compute(
                    "AllReduce", AOT.add,
                    replica_groups=[list(range(NC))],
                    ins=[din.opt()], outs=[dout.opt()],
                )
                return (g, dout)

            def stats_ar(li, sub, parts, nch, sums_t, sqs_t, lo, hi):
                st = pool.tile([parts, 2 * nch], F32, tag=f"st{sub}{li}", name=f"st{sub}{li}")
                nc.vector.tensor_reduce(st[:, 0:nch],
                                        sums_t[:, :, lo:hi] if nch > 1 else sums_t[:, lo:hi],
                                        mybir.AxisListType.X, AOT.add)
                nc.vector.tensor_reduce(st[:, nch:],
                                        sqs_t[:, :, lo:hi] if nch > 1 else sqs_t[:, lo:hi],
                                        mybir.AxisListType.X, AOT.add)
                return allreduce(st, parts, 2 * nch, f"{sub}{li}")

            def finish_stats(li, parts, nch, ga, gb, Gt, Bbt):
                for t_, d_ in (ga, gb):
                    nc.scalar.dma_start(t_[:], d_[:])
                g = pool.tile([parts, 2 * nch], F32, tag=f"g{li}", name=f"g{li}")
                nc.vector.tensor_tensor(g[:], ga[0][:], gb[0][:], AOT.add)
                return bn_coeffs(li, g, nch, parts, Gt, Bbt)

            NSPLIT = 15  # AR_a covers stat blocks [0,15), AR_b block 15

            _yb = {}

            def prefetch_y(li, nn):
                t_ = ybufp.tile([128, NCHUNK, 512], F32, tag="ybuf", name="ybuf")
                nc.sync.dma_start(t_[:], y_dram[li][:, :, nn * 512:(nn + 1) * 512])
                _yb[nn] = t_

            # ================ LAYER 1 matmul ================
            ar1a = None
            for n in range(NBLK):
                cols = slice(n * 512, (n + 1) * 512)
                psl = [ps1.tile([128, 512], F32, tag=f"ps_m{m}", name=f"ps_m{m}")
                       for m in range(NCHUNK)]
                rhs_tiles = []
                for k, (k0, kk) in enumerate(KJ):
                    xt_t = xpool.tile([128, 512], F32R, tag="xstream", name="xstream")
                    nc.sync.dma_start(xt_t[:kk, :], xT[k0:k0 + kk, cols])
                    rhs_tiles.append(xt_t[:kk, :])
                for k in range(len(KJ)):
                    for m in range(NCHUNK):
                        nc.tensor.matmul(psl[m][:], w1_sb[k][m][:], rhs_tiles[k],
                                         start=(k == 0), stop=(k == len(KJ) - 1))
                st_t = stage.tile([128, NCHUNK, 512], F32, tag="ystage", name="ystage")
                for m in range(NCHUNK):
                    nc.scalar.activation(st_t[:, m, :], psl[m][:], AFT.Copy,
                                         accum_out=sums[0][:, m, n:n + 1])
                    sc = scr.tile([128, 512], BF16, tag="sq_scratch", name="sq_scratch")
                    nc.scalar.activation(sc[:], psl[m][:], AFT.Square,
                                         accum_out=sqs[0][:, m, n:n + 1])
                nc.sync.dma_start(y_dram[0][:, :, cols], st_t[:])
                if n == 0:
                    load_params()
                if n == NSPLIT - 1:
                    ar1a = stats_ar(0, "a", 128, NCHUNK, sums[0], sqs[0], 0, NSPLIT)
            for nn in range(3):
                prefetch_y(0, nn)
            ar1b = stats_ar(0, "b", 128, NCHUNK, sums[0], sqs[0], NSPLIT, NBLK)
            c0_1, c1_1 = finish_stats(0, 128, NCHUNK, ar1a, ar1b, Gl[0], Bbl[0])

            # ---------------- LIF scan ----------------
            def fold_chunk(nn, m, c0, c1):
                """In place: yb[m] = c0 * y[m] + c1 (one ACT instr)."""
                yb = _yb[nn]
                nc.scalar.activation(yb[:, m, :], yb[:, m, :], AFT.Identity,
                                     bias=c1[:, m:m + 1], scale=c0[:, m:m + 1])

            # every ACT insert between Signs is a single instruction, so the
            # NPAR-step WAR window always absorbs it; drain j (chunk j//2 of
            # the m-major interleaved matmul) lands after its psum completes
            DRAIN_TT = {6: 0, 8: 1, 11: 2, 13: 3, 16: 4, 18: 5, 21: 6, 23: 7}
            FOLD_TT = {24: 0, 26: 1, 28: 2, 30: 3}
            WARM_TT = {12: 0, 14: 1, 17: 2, 19: 3}  # fold(1) during block 0

            def lif_scan(li, c0, c1, mm_cb, drain_cb):
                """512-step LIF scan; 3 DVE ops/step x 2 interleaved chains,
                Sign batched per NPAR steps on ACT.

                mm_cb(n): next-layer matmuls for block n (PE queue).
                drain_cb(n, j): j-th psum drain sub-step for block n (ACT),
                spread one instruction at a time through block n+1.
                """
                nbv = nbl[li][:].rearrange("p (c b) -> p c b", c=NCHUNK)
                nc.gpsimd.memset(Wt[:], 0.0)
                for m in range(NCHUNK):
                    fold_chunk(0, m, c0, c1)
                for n in range(NBLK):
                    if n + 3 < NBLK:
                        prefetch_y(li, n + 3)
                    yb = _yb.pop(n)
                    for tt in range(TBLK):
                        t = n * TBLK + tt
                        p = t % NPAR
                        bs = slice(tt * BL, (tt + 1) * BL)
                        # two independent chunk-chains interleaved so the
                        # DVE pipelines (no back-to-back RAW drain stalls)
                        Ua = Uq[:, 0:2, p, :]
                        Ub = Uq[:, 2:4, p, :]
                        nc.vector.tensor_tensor(Ua, Wt[:, 0:2, :], nbv[:, 0:2, :], AOT.mult)
                        nc.vector.tensor_tensor(Ub, Wt[:, 2:4, :], nbv[:, 2:4, :], AOT.mult)
                        nc.vector.tensor_tensor(Ua, Ua, yb[:, 0:2, bs], AOT.add)
                        nc.vector.tensor_tensor(Ub, Ub, yb[:, 2:4, bs], AOT.add)
                        nc.vector.scalar_tensor_tensor(Wt[:, 0:2, :], Ua, 1.0, Ua,
                                                       AOT.is_gt, AOT.subtract)
                        nc.vector.scalar_tensor_tensor(Wt[:, 2:4, :], Ub, 1.0, Ub,
                                                       AOT.is_gt, AOT.subtract)
                        if p == NPAR - 1:
                            # S'[cols t-7..t] = Sign(U[0..7] - 1), one ACT op
                            # (contiguous 128-col run per chunk on both sides)
                            t0 = t - (NPAR - 1)
                            sdst = S[:, :, t0 * BL:(t0 + NPAR) * BL]
                            usrc = Uq[:].rearrange("p c q b -> p c (q b)")
                            nc.scalar.activation(sdst, usrc, AFT.Sign,
                                                 bias=negone[:])
                        if drain_cb is not None and n > 0 and tt in DRAIN_TT:
                            drain_cb(n - 1, DRAIN_TT[tt])
                        if n == 0 and tt in WARM_TT:
                            fold_chunk(1, WARM_TT[tt], c0, c1)
                        if n + 2 < NBLK and tt in FOLD_TT:
                            fold_chunk(n + 2, FOLD_TT[tt], c0, c1)
                    if mm_cb is not None:
                        mm_cb(n)
                if drain_cb is not None:
                    for j in range(8):
                        drain_cb(NBLK - 1, j)

            # scan1 + interleaved L2 matmul
            _l2ps = {}

            def l2_mm_cb(n):
                cols = slice(n * 512, (n + 1) * 512)
                psl = [ps1.tile([128, 512], F32, tag=f"ps_m{m}", name=f"ps_m{m}")
                       for m in range(NCHUNK)]
                for m in range(NCHUNK):  # m-major: chunk psums finish early
                    for k in range(NCHUNK):
                        nc.tensor.matmul(psl[m][:], w2_sb[k][m][:], S[:, k, cols],
                                         start=(k == 0), stop=(k == NCHUNK - 1))
                _l2ps[n] = psl

            ar2a = None
            _l2st = {}

            def l2_drain_cb(n, j):
                nonlocal ar2a
                m = j // 2
                if j == 0:
                    _l2st[n] = stage.tile([128, NCHUNK, 512], F32, tag="ystage", name="ystage")
                st_t = _l2st[n]
                if j % 2 == 0:
                    nc.scalar.activation(st_t[:, m, :], _l2ps[n][m][:], AFT.Copy,
                                         accum_out=sums[1][:, m, n:n + 1])
                else:
                    sc = scr.tile([128, 512], BF16, tag="sq_scratch", name="sq_scratch")
                    nc.scalar.activation(sc[:], _l2ps[n][m][:], AFT.Square,
                                         accum_out=sqs[1][:, m, n:n + 1])
                if j == 7:
                    _l2ps.pop(n)
                    _l2st.pop(n)
                    nc.sync.dma_start(y_dram[1][:, :, n * 512:(n + 1) * 512], st_t[:])
                    if n == NSPLIT - 1:
                        ar2a = stats_ar(1, "a", 128, NCHUNK, sums[1], sqs[1], 0, NSPLIT)

            lif_scan(0, c0_1, c1_1, l2_mm_cb, l2_drain_cb)
            for nn in range(3):
                prefetch_y(1, nn)
            ar2b = stats_ar(1, "b", 128, NCHUNK, sums[1], sqs[1], NSPLIT, NBLK)
            c0_2, c1_2 = finish_stats(1, 128, NCHUNK, ar2a, ar2b, Gl[1], Bbl[1])

            # scan2 + interleaved readout matmul
            _l3ps = {}

            def l3_mm_cb(n):
                cols = slice(n * 512, (n + 1) * 512)
                ps = ps1.tile([O, 512], F32, tag="ps_m0", name="ps_r")
                for k in range(NCHUNK):
                    nc.tensor.matmul(ps[:], wr_sb[k][:], S[:, k, cols],
                                     start=(k == 0), stop=(k == NCHUNK - 1))
                _l3ps[n] = ps

            ar3a = None

            def l3_drain_cb(n, j):
                nonlocal ar3a
                cols = slice(n * 512, (n + 1) * 512)
                if j == 0:
                    nc.scalar.activation(y3[:, cols], _l3ps[n][:], AFT.Copy,
                                         accum_out=sumr[:, n:n + 1])
                elif j == 1:
                    sc = scr.tile([O, 512], BF16, tag="sq3_scratch", name="sq3_scratch")
                    nc.scalar.activation(sc[:], _l3ps[n][:], AFT.Square,
                                         accum_out=sqr[:, n:n + 1])
                    _l3ps.pop(n)
                    if n == NSPLIT - 1:
                        ar3a = stats_ar(2, "a", O, 1, sumr, sqr, 0, NSPLIT)

            lif_scan(1, c0_2, c1_2, l3_mm_cb, l3_drain_cb)
            ar3b = stats_ar(2, "b", O, 1, sumr, sqr, NSPLIT, NBLK)
            c0_r, c1_r = finish_stats(2, O, 1, ar3a, ar3b, Gr_sb, Bbr_sb)

            # ================ READOUT ================
            # BN-fold y3 in place (4 slabs for pipelining)
            for q in range(4):
                sl = slice(q * 2048, (q + 1) * 2048)
                nc.scalar.activation(y3[:, sl], y3[:, sl], AFT.Identity,
                                     bias=c1_r[:, 0:1], scale=c0_r[:, 0:1])
            # leaky-integrator scans, in place (fp32)
            y3v = y3[:].rearrange("p (t b) -> p t b", b=BL)
            for b in range(BL):
                sl = y3v[:, :, b]
                nc.vector.tensor_tensor_scan(sl, brb_sb[:], sl, 0.0, AOT.mult, AOT.add)

            # softmax over channels, in place; then T-sum.
            # Phase 1: Z rows for all 16 blocks packed into one [16,512]
            # psum via PE accumulation; single exact reciprocal.
            ones_k20 = pool.tile([O, 1], F16, tag="ones_k20", name="ones_k20")
            nc.gpsimd.memset(ones_k20[:], 1.0)
            # E[0, n*16+m] = (n == m): row selectors for the Z-pack matmuls
            Epack = pool.tile([1, NBLK * NBLK], F16, tag="Epack", name="Epack")
            nc.sync.dma_start(Epack[:], epack_d)
            # Sel[k, n*20:(n+1)*20] = (k == n): selects Z row n, bcast to 20
            Sel = pool.tile([NBLK, NBLK * O], F16, tag="Sel", name="Sel")
            nc.sync.dma_start(Sel[:], sel_d)
            zall_ps = ps1.tile([NBLK, 512], F32, tag="ps_m1", name="ps_zall")
            for n in range(NBLK):
                cols = slice(n * 512, (n + 1) * 512)
                En = scr.tile([O, 512], F16, tag="En", name="En")
                nc.scalar.activation(En[:], y3[:, cols], AFT.Exp)
                psz = ps1.tile([1, 512], F32, tag="ps_m2", name="ps_z")
                nc.tensor.matmul(psz[:], ones_k20[:], En[:], start=True, stop=True)
                zsb = scr.tile([1, 512], F16, tag="zsb", name="zsb")
                nc.scalar.copy(zsb[:], psz[:])
                nc.tensor.matmul(zall_ps[:], Epack[0:1, n * NBLK:(n + 1) * NBLK],
                                 zsb[:], start=(n == 0), stop=(n == NBLK - 1))
            Rall = pool.tile([NBLK, 512], F16, tag="Rall", name="Rall")
            with nc.allow_low_precision(reason="softmax denominator, fp16 ok"):
                nc.vector.reciprocal(Rall[:], zall_ps[:])
            # Phase 2: broadcast 1/Z to 20 partitions per block, multiply.
            for n in range(NBLK):
                cols = slice(n * 512, (n + 1) * 512)
                En = scr.tile([O, 512], F16, tag="En", name="En")
                nc.scalar.activation(En[:], y3[:, cols], AFT.Exp)
                psb = ps1.tile([O, 512], F32, tag="ps_m3", name="ps_b")
                nc.tensor.matmul(psb[:], Sel[:, n * O:(n + 1) * O], Rall[:],
                                 start=True, stop=True)
                nc.vector.tensor_tensor(y3[:, cols], En[:], psb[:], AOT.mult)
            # T-sum: view [O, b, t] -> reduce over t
            res = pool.tile([O, BL], F32, tag="res", name="res")
            accv = y3[:].rearrange("p (t b) -> p b t", b=BL)
            nc.vector.tensor_reduce(res[:, 0:BL // 2], accv[:, 0:BL // 2, :],
                                    mybir.AxisListType.X, AOT.add)
            nc.vector.tensor_reduce(res[:, BL // 2:], accv[:, BL // 2:, :],
                                    mybir.AxisListType.X, AOT.add)
            nc.sync.dma_start(out_d, res[:])

    nc.compile()
    return nc


def _host_prep(inputs):
    f32 = np.float32
    x = np.asarray(inputs["x"], f32)
    sig = lambda v: (1.0 / (1.0 + np.exp(-np.asarray(v, np.float64)))).astype(f32)

    def packed(vec):  # [H] -> [128, NCHUNK]
        return np.ascontiguousarray(np.asarray(vec, f32).reshape(NCHUNK, 128).T)

    beta1, beta2, betar = sig(inputs["beta1"]), sig(inputs["beta2"]), sig(inputs["betar"])

    def nbcast(beta):  # [H] -> [128, 64] = -beta, chunk-major, bcast over b
        p = packed(-beta)  # [128, 4]
        return np.ascontiguousarray(np.repeat(p[:, :, None], BL, axis=2).reshape(128, NCHUNK * BL))

    com = {
        "w1t": np.ascontiguousarray(np.asarray(inputs["W1"], f32).T),
        "w2t": np.ascontiguousarray(np.asarray(inputs["W2"], f32).T * 0.5).astype(ml_dtypes.bfloat16),
        "wrt": np.ascontiguousarray(np.asarray(inputs["Wr"], f32).T * 0.5).astype(ml_dtypes.bfloat16),
        "nb64_1": nbcast(beta1),
        "nb64_2": nbcast(beta2),
        "G1": packed(np.asarray(inputs["g1"], f32) * (1 - beta1)),
        "Bb1": packed(np.asarray(inputs["b1"], f32) * (1 - beta1)),
        "G2": packed(np.asarray(inputs["g2"], f32) * (1 - beta2)),
        "Bb2": packed(np.asarray(inputs["b2"], f32) * (1 - beta2)),
        "brb": np.ascontiguousarray(np.repeat(betar[:, None], 512, axis=1)),
        "Gr": np.ascontiguousarray((np.asarray(inputs["gr"], f32) * (1 - betar))[:, None]),
        "Bbr": np.ascontiguousarray((np.asarray(inputs["br"], f32) * (1 - betar))[:, None]),
        "epack": np.eye(NBLK, dtype=np.float16).reshape(1, NBLK * NBLK),
        "sel": np.ascontiguousarray(
            np.repeat(np.eye(NBLK, dtype=np.float16)[:, :, None], O, axis=2).reshape(NBLK, NBLK * O)),
    }
    in_maps = []
    for c in range(NC):
        xc = x[c * BL:(c + 1) * BL]              # [BL, T, J]
        xTc = np.ascontiguousarray(xc.transpose(2, 1, 0).reshape(J, NCOL))
        m = dict(com)
        m["xT"] = xTc
        in_maps.append(m)
    return in_maps


def kernel(**inputs):
    if "nc" not in _CACHE:
        _CACHE["nc"] = _build_program()
    nc = _CACHE["nc"]
    in_maps = _host_prep(inputs)
    res = run_bass_kernel_spmd(nc, in_maps, core_ids=list(range(NC)),
                               trace=bool(os.environ.get("BASS_TRACE_KERNEL")))
    _CACHE["last_result"] = res
    out = np.empty((B, O), np.float32)
    for c in range(NC):
        out[c * BL:(c + 1) * BL, :] = res.results[c]["out"].T
    return out


if __name__ == "__main__":
    t0 = time.time()
    nc = _build_program()
    print(f"build+compile ok in {time.time()-t0:.1f}s")
